# revision 4
# baseline (speedup 1.0000x reference)
"""BiLSTM classifier Trainium2 kernel, v3.

Data-parallel over batch across 8 NeuronCores (BL=32/core, replicated
weights). Single-core program per core; no collectives.

Layout (walrus-valid: DoubleRow matmuls must write PSUM partition 0):
- L1 gates per quarter in PSUM [32, 1024]: cols 0:512 fwd, 512:1024 bwd.
  One activation covers both directions; c-chain is [32, 1024].
- L2 gates per quarter in PSUM [32, 1024]: cols = two hidden halves.
- Per-gate emission order [f, g, i, o]: sigmoid(f) -> v=f*c starts
  while later gate matmuls still stream.
- h^T history kept in SBUF per-m-tile ring tiles (hTs); Xs phase reads
  its stationary operands straight from them (no DRAM round trip).
- bwd-direction X precompute emitted in descending x-mt order so the
  bwd recurrence (which consumes x reversed) starts immediately.
- ih/A/Xs/B emitted interleaved (software pipeline) so the Tile
  scheduler sees instructions in near-execution order.
- hh/ws weights fp8-e4m3 DoubleRow (0.5 cyc/row); ih bf16; gate
  activations fp16 (DVE 2x where eligible); c-state f32.
"""

import sys

sys.path.insert(0, "/opt/trn_rl_repo")

import numpy as np
import ml_dtypes

import concourse.bass as bass
import concourse.mybir as mybir
import concourse.tile as tile
from concourse import bacc
from concourse.bass_utils import run_bass_kernel_spmd

AF = mybir.ActivationFunctionType
BF16 = mybir.dt.bfloat16
F16 = mybir.dt.float16
F32 = mybir.dt.float32
F8 = mybir.dt.float8e4
DR = mybir.MatmulPerfMode.DoubleRow

B, D, H = 256, 256, 512
H2 = 2 * H
G1 = 4 * H           # 2048 = L1 gate width per direction
G2 = 4 * H2          # 4096 = L2 gate width
L = 2
NCORES = 8
BL = B // NCORES     # 32 local batch

# gate-quarter emission order: f first (so v=f*c starts early), o last
QORDER = (2, 0, 1, 3)  # col order is [g|i|f|o]


def _ih_iter(nc, pools, T, t_, g, i):
    """One ih iteration: Xf[i] (fwd) + Xb[MT-1-i] (bwd, descending)."""
    MT = T // 4
    xT = t_["xT"]
    ihx, ihp, iho = pools["ihx"], pools["ihp"], pools["iho"]
    for d in ("f", "b"):
        mt = i if d == "f" else MT - 1 - i
        t0 = mt * 4
        xt = ihx.tile([128, 2, 4, 32], BF16, tag=f"xt_{d}", name=f"xt_{d}")
        nc.sync.dma_start(
            out=xt,
            in_=xT.rearrange("(k p) t b -> p k t b", p=128)[:, :, t0:t0 + 4, :],
        )
        Xd = g["Xf"] if d == "f" else g["Xb"]
        wih = t_[f"w{d}_ih_sb"]
        bih = t_[f"b{d}_sb"]
        for nh in range(4):
            nsl = slice(nh * 512, (nh + 1) * 512)
            ps = ihp.tile([128, 512], F32, tag="ps", name="ps")
            for kt in range(2):
                nc.tensor.matmul(
                    ps, xt[:, kt].rearrange("p t b -> p (t b)"),
                    wih[:, kt, nsl], start=(kt == 0), stop=False,
                )
            nc.tensor.matmul(
                ps, t_["ones_t"][:, :128], bih[:, nsl],
                start=False, stop=True,
            )
            ot = iho.tile([128, 512], BF16, tag="ot", name="ot")
            nc.vector.tensor_copy(ot, ps)
            nc.sync.dma_start(
                out=Xd[mt][:, :, nsl].rearrange("t b n -> (t b) n"),
                in_=ot,
            )


def _a_step(nc, pools, T, t_, g, s):
    """One L1 step; fwd at cols 0:512, bwd at cols 512:1024 per quarter."""
    ax, ag, atr, ah, aact = (pools[k] for k in ("ax", "ag", "atr", "ah", "aact"))
    c1 = g["c1"]
    if s % 4 == 0:
        # [128, dir, kt, st, b] fp8 - per-m-tile h^T history ring
        g["hTs"][s // 4] = ah.tile(
            [128, 2, 4, 4, 32], F8, tag="hTs", name=f"hTs{s//4}")
    hTs = g["hTs"][s // 4]

    def prev_ap(bi, kp):
        if s == 0:
            return g["hT0"][:, bi, 2 * kp:2 * kp + 2, :]
        return g["hTs"][(s - 1) // 4][:, bi, 2 * kp:2 * kp + 2, (s - 1) % 4, :]

    xfb = ax.tile([32, 2 * G1], BF16, tag="xfb", name="xfb")
    nc.sync.dma_start(out=xfb[:, 0:G1], in_=g["Xf"][s // 4][s % 4])
    sb = T - 1 - s
    nc.sync.dma_start(out=xfb[:, G1:2 * G1], in_=g["Xb"][sb // 4][sb % 4])

    acts = {}
    for q in QORDER:
        nm = {2: "sf", 0: "tg", 1: "si", 3: "so"}[q]
        acts[nm] = aact.tile([32, 1024], F16, tag=nm, name=nm)
    for q in QORDER:
        nm = {2: "sf", 0: "tg", 1: "si", 3: "so"}[q]
        fn = AF.Tanh if q == 0 else AF.Sigmoid
        for bi, d in enumerate(("f", "b")):
            gp = ag.tile([32, 512], F32, tag="gq", name=f"gq{q}{d}")
            nc.tensor.matmul(
                gp, t_["id32_t"],
                xfb[:, bi * G1 + q * 512: bi * G1 + (q + 1) * 512],
                start=True, stop=False,
            )
            whh = t_[f"w{d}_hh_sb"]
            for kp in range(2):
                nc.tensor.matmul(
                    gp, prev_ap(bi, kp),
                    whh[:, 2 * kp:2 * kp + 2, q * 512:(q + 1) * 512],
                    start=False, stop=(kp == 1), perf_mode=DR,
                )
            nc.scalar.activation(
                acts[nm][:, 512 * bi:512 * bi + 512], gp, fn)
    sf, tg, si, so = acts["sf"], acts["tg"], acts["si"], acts["so"]

    v = aact.tile([32, 1024], F16, tag="v", name="v")
    nc.vector.tensor_mul(v, sf, c1)
    u = aact.tile([32, 1024], F16, tag="u", name="u")
    nc.vector.tensor_mul(u, si, tg)
    nc.vector.tensor_add(c1, u, v)
    tcl = aact.tile([32, 1024], F16, tag="tc", name="tc")
    nc.scalar.activation(tcl, c1, AF.Tanh)
    h = aact.tile([32, 1024], BF16, tag="h", name="h")
    nc.vector.tensor_mul(h, so, tcl)

    ptr = atr.tile([128, 8, 32], BF16, tag="ptr", name="ptr")
    for ci in range(8):  # cols 0:512 fwd (kt=ci), 512:1024 bwd (kt=ci-4)
        nc.tensor.transpose(
            ptr[:, ci], h[:, ci * 128:(ci + 1) * 128], t_["id32_t"],
        )
    nc.vector.tensor_copy(
        hTs[:, :, :, s % 4, :], ptr.rearrange("p (d k) b -> p d k b", d=2))


def _xs_mt(nc, pools, T, t_, g, mt):
    """Xs[mt] = combined @ Ws_ih^T + bs (natural col order)."""
    sp, so_ = pools["sp"], pools["so_"]
    wsih = t_["ws_ih_sb"]
    hTs = g["hTs"][mt]
    for chunk in range(8):
        csl = slice(chunk * 512, (chunk + 1) * 512)
        ps = sp.tile([128, 512], F32, tag="ps", name="ps")
        # stationary k-slab pairs: (f0,f1),(f2,f3),(b0,b1),(b2,b3)
        for kp in range(4):
            bi = kp // 2
            ko = (kp % 2) * 2
            lhs = hTs[:, bi, ko:ko + 2, :, :].rearrange("p k t b -> p k (t b)")
            nc.tensor.matmul(
                ps, lhs, wsih[:, 2 * kp:2 * kp + 2, csl],
                start=(kp == 0), stop=False, perf_mode=DR,
            )
        nc.tensor.matmul(
            ps, t_["ones_t"][:, :128], t_["bs_sb"][:, csl],
            start=False, stop=True,
        )
        ot = so_.tile([128, 512], BF16, tag="ot", name="ot")
        nc.vector.tensor_copy(ot, ps)
        nc.sync.dma_start(
            out=g["Xs"][mt][:, :, csl].rearrange("t b n -> (t b) n"),
            in_=ot,
        )


def _b_step(nc, pools, T, t_, g, s):
    """One L2 step; hidden halves at cols 0:512 / 512:1024 per quarter."""
    bx, bg, btr, bact = (pools[k] for k in ("bx", "bg", "btr", "bact"))
    wshh = t_["ws_hh_sb"]
    h2T, c2 = g["h2T"], g["c2"]
    xs2 = bx.tile([32, G2], BF16, tag="xs2", name="xs2")
    nc.sync.dma_start(out=xs2, in_=g["Xs"][s // 4][s % 4])
    acts = {}
    for q in QORDER:
        nm = {2: "sf2", 0: "tg2", 1: "si2", 3: "so2"}[q]
        acts[nm] = bact.tile([32, 1024], F16, tag=nm, name=nm)
    for q in QORDER:
        nm = {2: "sf2", 0: "tg2", 1: "si2", 3: "so2"}[q]
        fn = AF.Tanh if q == 0 else AF.Sigmoid
        for j in range(2):
            wsl = slice(1024 * q + 512 * j, 1024 * q + 512 * j + 512)
            gp = bg.tile([32, 512], F32, tag="gq2", name=f"gq2_{q}{j}")
            nc.tensor.matmul(
                gp, t_["id32_t"], xs2[:, wsl],
                start=True, stop=False,
            )
            for kp in range(4):
                nc.tensor.matmul(
                    gp, h2T[:, 2 * kp:2 * kp + 2, :],
                    wshh[:, 2 * kp:2 * kp + 2, wsl],
                    start=False, stop=(kp == 3), perf_mode=DR,
                )
            nc.scalar.activation(
                acts[nm][:, 512 * j:512 * j + 512], gp, fn)
    sf, tg, si, so = acts["sf2"], acts["tg2"], acts["si2"], acts["so2"]

    v = bact.tile([32, 1024], F16, tag="v2", name="v2")
    nc.vector.tensor_mul(v, sf, c2)
    u = bact.tile([32, 1024], F16, tag="u2", name="u2")
    nc.vector.tensor_mul(u, si, tg)
    nc.vector.tensor_add(c2, u, v)
    tcl = bact.tile([32, 1024], F16, tag="tc2", name="tc2")
    nc.scalar.activation(tcl, c2, AF.Tanh)
    h2 = bact.tile([32, 1024], BF16, tag="h2", name="h2")
    nc.vector.tensor_mul(h2, so, tcl)

    ptr = btr.tile([128, 8, 32], BF16, tag="ptr2", name="ptr2")
    for ci in range(8):  # cols are h2-dims in natural (slab) order
        nc.tensor.transpose(
            ptr[:, ci], h2[:, ci * 128:(ci + 1) * 128], t_["id32_t"],
        )
    nc.vector.tensor_copy(h2T, ptr)


def _classifier(nc, pools, t_, g):
    bg, bact = pools["bg"], pools["bact"]
    h2b = bact.tile([128, 8, 32], BF16, tag="h2b", name="h2b")
    nc.vector.tensor_copy(h2b, g["h2T"])
    ps_full = bg.tile([32, 512], F32, tag="gq2", name="ps_cls")
    ps_o = ps_full[0:BL, 0:L]
    for kt in range(8):
        nc.tensor.matmul(
            ps_o, h2b[:, kt], t_["wl_sb"][:, kt],
            start=(kt == 0), stop=False,
        )
    nc.tensor.matmul(
        ps_o, t_["ones_t"][:, :BL], t_["bl_sb"], start=False, stop=True)
    o_sb = bact.tile([BL, L], F32, tag="o_sb", name="o_sb")
    nc.scalar.activation(o_sb, ps_o, AF.Sigmoid)
    nc.sync.dma_start(out=t_["out"][:, :], in_=o_sb)


def _build_nc(T: int, use_fp8=True):
    assert use_fp8, "v3 kernel is fp8-DR only"
    nc = bacc.Bacc(None, target_bir_lowering=False)

    t_ = {}
    t_["xT"] = nc.dram_tensor("xT", [D, T, BL], BF16, kind="ExternalInput")
    for name, shape in (
        ("wf_ih", [D, G1]), ("wb_ih", [D, G1]),
        ("bf_r", [1, G1]), ("bb_r", [1, G1]), ("bs_r", [1, G2]),
        ("wl", [H2, L]), ("bl_r", [1, L]),
        ("ones_r", [1, 128]), ("id32", [32, 32]),
    ):
        t_[name] = nc.dram_tensor(name, shape, BF16, kind="ExternalInput")
    for name, shape in (
        ("wf_hh", [H, G1]), ("wb_hh", [H, G1]),
        ("ws_hh", [H2, G2]), ("ws_ih", [H2, G2]),
    ):
        t_[name] = nc.dram_tensor(name, shape, F8, kind="ExternalInput")
    t_["out"] = nc.dram_tensor("out", [BL, L], F32, kind="ExternalOutput")

    with tile.TileContext(nc) as tc:
        from contextlib import ExitStack
        with ExitStack() as ctx:
            ec = ctx.enter_context
            dram = ec(tc.tile_pool(name="dram", bufs=1, space="DRAM"))
            const = ec(tc.tile_pool(name="const", bufs=1))
            wpool = ec(tc.tile_pool(name="wpool", bufs=1))
            state = ec(tc.tile_pool(name="state", bufs=1))

            MT = T // 4
            g = {"hTs": {}}
            g["Xf"] = [dram.tile([4, BL, G1], BF16, tag=f"Xf{m}", name=f"Xf{m}") for m in range(MT)]
            g["Xb"] = [dram.tile([4, BL, G1], BF16, tag=f"Xb{m}", name=f"Xb{m}") for m in range(MT)]
            g["Xs"] = [dram.tile([4, BL, G2], BF16, tag=f"Xs{m}", name=f"Xs{m}") for m in range(MT)]

            ones_t = const.tile([1, 128], BF16, tag="ones_t", name="ones_t")
            nc.sync.dma_start(out=ones_t, in_=t_["ones_r"][:, :])
            id32_t = const.tile([32, 32], BF16, tag="id32_t", name="id32_t")
            nc.sync.dma_start(out=id32_t, in_=t_["id32"][:, :])
            t_["ones_t"], t_["id32_t"] = ones_t, id32_t

            # weights to SBUF
            for d in ("f", "b"):
                w = wpool.tile([128, 2, G1], BF16, tag=f"w{d}ih", name=f"w{d}ih")
                nc.sync.dma_start(out=w, in_=t_[f"w{d}_ih"].rearrange("(k p) n -> p k n", p=128))
                t_[f"w{d}_ih_sb"] = w
                w = wpool.tile([128, 4, G1], F8, tag=f"w{d}hh", name=f"w{d}hh")
                nc.sync.dma_start(out=w, in_=t_[f"w{d}_hh"].rearrange("(k p) n -> p k n", p=128))
                t_[f"w{d}_hh_sb"] = w
                bt = wpool.tile([1, G1], BF16, tag=f"b{d}", name=f"b{d}")
                nc.sync.dma_start(out=bt, in_=t_[f"b{d}_r"][:, :])
                t_[f"b{d}_sb"] = bt
            for nm in ("ws_ih", "ws_hh"):
                w = wpool.tile([128, 8, G2], F8, tag=nm, name=nm)
                nc.sync.dma_start(out=w, in_=t_[nm].rearrange("(k p) n -> p k n", p=128))
                t_[f"{nm}_sb"] = w
            bs = wpool.tile([1, G2], BF16, tag="bs", name="bs")
            nc.sync.dma_start(out=bs, in_=t_["bs_r"][:, :])
            t_["bs_sb"] = bs
            wl = wpool.tile([128, 8, L], BF16, tag="wl", name="wl")
            nc.sync.dma_start(out=wl, in_=t_["wl"].rearrange("(k p) n -> p k n", p=128))
            t_["wl_sb"] = wl
            bl = wpool.tile([1, L], BF16, tag="bl", name="bl")
            nc.sync.dma_start(out=bl, in_=t_["bl_r"][:, :])
            t_["bl_sb"] = bl

            g["hT0"] = state.tile([128, 2, 4, 32], F8, tag="hT0", name="hT0")
            nc.vector.memset(g["hT0"], 0.0)
            g["c1"] = state.tile([32, 1024], F16, tag="c1", name="c1")
            nc.vector.memset(g["c1"], 0.0)
            g["h2T"] = state.tile([128, 8, 32], F8, tag="h2T", name="h2T")
            nc.vector.memset(g["h2T"], 0.0)
            g["c2"] = state.tile([32, 1024], F16, tag="c2", name="c2")
            nc.vector.memset(g["c2"], 0.0)

            pools = {}
            pools["ihx"] = ec(tc.tile_pool(name="ihx", bufs=3))
            pools["ihp"] = ec(tc.tile_pool(name="ihp", bufs=1, space="PSUM"))
            pools["iho"] = ec(tc.tile_pool(name="iho", bufs=3))
            pools["ax"] = ec(tc.tile_pool(name="ax", bufs=2))
            pools["ag"] = ec(tc.tile_pool(name="ag", bufs=2, space="PSUM"))
            pools["atr"] = ec(tc.tile_pool(name="atr", bufs=1, space="PSUM"))
            pools["ah"] = ec(tc.tile_pool(name="ah", bufs=3))
            pools["aact"] = ec(tc.tile_pool(name="aact", bufs=1))
            pools["sp"] = ec(tc.tile_pool(name="sp", bufs=1, space="PSUM"))
            pools["so_"] = ec(tc.tile_pool(name="so_", bufs=3))
            pools["bx"] = ec(tc.tile_pool(name="bx", bufs=2))
            pools["bg"] = ec(tc.tile_pool(name="bg", bufs=2, space="PSUM"))
            pools["btr"] = ec(tc.tile_pool(name="btr", bufs=1, space="PSUM"))
            pools["bact"] = ec(tc.tile_pool(name="bact", bufs=1))

            # software pipeline: ih(i) || A(4(i-1)..) || Xs(i-2) || B(4(i-2)..)
            for i in range(MT + 2):
                if i < MT:
                    _ih_iter(nc, pools, T, t_, g, i)
                if 1 <= i <= MT:
                    for st in range(4):
                        _a_step(nc, pools, T, t_, g, 4 * (i - 1) + st)
                if 2 <= i:
                    mt = i - 2
                    _xs_mt(nc, pools, T, t_, g, mt)
                    for st in range(4):
                        _b_step(nc, pools, T, t_, g, 4 * mt + st)
            _classifier(nc, pools, t_, g)
    nc.compile()
    return nc


_NC_CACHE = {}
USE_FP8 = True


def _get_nc(T, use_fp8=None):
    if T not in _NC_CACHE:
        _NC_CACHE[T] = _build_nc(T)
    return _NC_CACHE[T]


def _bf16(a):
    return np.ascontiguousarray(np.asarray(a, dtype=np.float32)).astype(ml_dtypes.bfloat16)


def _f8(a):
    a = np.clip(np.asarray(a, dtype=np.float32), -240.0, 240.0)
    return np.ascontiguousarray(a).astype(ml_dtypes.float8_e4m3)


def _prep_weights(Wf_ih, Wf_hh, bf, Wb_ih, Wb_hh, bb, Ws_ih, Ws_hh, bs, Wl, bl):
    # gate reorder [i|f|g|o] -> [g|i|f|o]
    r1 = np.r_[2 * H:3 * H, 0:H, H:2 * H, 3 * H:4 * H]
    r2 = np.r_[2 * H2:3 * H2, 0:H2, H2:2 * H2, 3 * H2:4 * H2]
    return {
        "wf_ih": _bf16(np.asarray(Wf_ih)[r1].T),
        "wf_hh": _f8(np.asarray(Wf_hh)[r1].T),
        "wb_ih": _bf16(np.asarray(Wb_ih)[r1].T),
        "wb_hh": _f8(np.asarray(Wb_hh)[r1].T),
        "ws_ih": _f8(np.asarray(Ws_ih)[r2].T),
        "ws_hh": _f8(np.asarray(Ws_hh)[r2].T),
        "bf_r": _bf16(np.asarray(bf)[r1][None, :]),
        "bb_r": _bf16(np.asarray(bb)[r1][None, :]),
        "bs_r": _bf16(np.asarray(bs)[r2][None, :]),
        "wl": _bf16(np.asarray(Wl).T),
        "bl_r": _bf16(np.asarray(bl)[None, :]),
        "ones_r": _bf16(np.ones((1, 128), np.float32)),
        "id32": _bf16(np.eye(32, dtype=np.float32)),
    }


def input_specs(T):
    return [
        ("xT", [D, T, BL], BF16), ("wf_ih", [D, G1], BF16),
        ("wb_ih", [D, G1], BF16), ("bf_r", [1, G1], BF16),
        ("bb_r", [1, G1], BF16), ("bs_r", [1, G2], BF16),
        ("wl", [H2, L], BF16), ("bl_r", [1, L], BF16),
        ("ones_r", [1, 128], BF16), ("id32", [32, 32], BF16),
        ("wf_hh", [H, G1], F8), ("wb_hh", [H, G1], F8),
        ("ws_hh", [H2, G2], F8), ("ws_ih", [H2, G2], F8),
    ]


def make_in_maps(inputs):
    x = np.asarray(inputs["x"], dtype=np.float32)
    wmap = _prep_weights(
        inputs["Wf_ih"], inputs["Wf_hh"], inputs["bf"],
        inputs["Wb_ih"], inputs["Wb_hh"], inputs["bb"],
        inputs["Ws_ih"], inputs["Ws_hh"], inputs["bs"],
        inputs["Wl"], inputs["bl"])
    in_maps = []
    for c in range(NCORES):
        m = dict(wmap)
        m["xT"] = _bf16(x[c * BL:(c + 1) * BL].transpose(2, 1, 0))
        in_maps.append(m)
    return in_maps


def kernel(x, Wf_ih, Wf_hh, bf, Wb_ih, Wb_hh, bb, Ws_ih, Ws_hh, bs, Wl, bl):
    x = np.asarray(x, dtype=np.float32)
    T = x.shape[1]
    nc = _get_nc(T)
    in_maps = make_in_maps(dict(
        x=x, Wf_ih=Wf_ih, Wf_hh=Wf_hh, bf=bf, Wb_ih=Wb_ih, Wb_hh=Wb_hh,
        bb=bb, Ws_ih=Ws_ih, Ws_hh=Ws_hh, bs=bs, Wl=Wl, bl=bl))
    res = run_bass_kernel_spmd(nc, in_maps, list(range(NCORES)))
    return np.concatenate([res.results[c]["out"] for c in range(NCORES)], axis=0)


if __name__ == "__main__":
    rng = np.random.default_rng(0)
    T = int(sys.argv[1]) if len(sys.argv) > 1 else 8
    ins = {
        "x": rng.standard_normal((B, T, D), dtype=np.float32),
        "Wf_ih": rng.standard_normal((4 * H, D), dtype=np.float32) * 0.05,
        "Wf_hh": rng.standard_normal((4 * H, H), dtype=np.float32) * 0.04,
        "bf": np.zeros(4 * H, np.float32),
        "Wb_ih": rng.standard_normal((4 * H, D), dtype=np.float32) * 0.05,
        "Wb_hh": rng.standard_normal((4 * H, H), dtype=np.float32) * 0.04,
        "bb": np.zeros(4 * H, np.float32),
        "Ws_ih": rng.standard_normal((4 * H2, H2), dtype=np.float32) * 0.03,
        "Ws_hh": rng.standard_normal((4 * H2, H2), dtype=np.float32) * 0.03,
        "bs": np.zeros(4 * H2, np.float32),
        "Wl": rng.standard_normal((L, H2), dtype=np.float32) * 0.04,
        "bl": np.zeros(L, np.float32),
    }
    got = kernel(**ins)

    def sigmoid(z):
        return 1.0 / (1.0 + np.exp(-z))

    def scan(xs, Wih, Whh, bvec):
        Tn, Bn, _ = xs.shape
        Hh = Whh.shape[1]
        h = np.zeros((Bn, Hh), np.float32)
        c = np.zeros((Bn, Hh), np.float32)
        hs = []
        for t in range(Tn):
            gg = xs[t] @ Wih.T + h @ Whh.T + bvec
            i, f, ge, o = np.split(gg, 4, axis=-1)
            c = sigmoid(f) * c + sigmoid(i) * np.tanh(ge)
            h = sigmoid(o) * np.tanh(c)
            hs.append(h.copy())
        return np.stack(hs), h

    xs = np.swapaxes(ins["x"], 0, 1)
    fseq, _ = scan(xs, ins["Wf_ih"], ins["Wf_hh"], ins["bf"])
    bseq, _ = scan(xs[::-1], ins["Wb_ih"], ins["Wb_hh"], ins["bb"])
    comb = np.concatenate([fseq, bseq], -1)
    _, hs = scan(comb, ins["Ws_ih"], ins["Ws_hh"], ins["bs"])
    ref = sigmoid(hs @ ins["Wl"].T + ins["bl"])
    rel = np.abs(got - ref) / np.maximum(np.abs(ref), 1e-6)
    print(f"T={T}: max rel {rel.max():.3e}  mean rel {rel.mean():.3e}")


# revision 6
# speedup vs baseline: 11.5626x; 11.5626x over previous
"""BiLSTM classifier Trainium2 kernel (final).

Data-parallel over batch across 8 NeuronCores (BL=32/core, replicated
weights). Single-core program per core; no collectives.

Layout (walrus-valid: DoubleRow matmuls must write PSUM partition 0):
- L1 gates per quarter in PSUM [32, 1024]: cols 0:512 fwd, 512:1024 bwd.
  One activation covers both directions; c-chain is [32, 1024].
- L2 gates per quarter in PSUM [32, 1024]: cols = two hidden halves.
- Per-gate emission order [f, g, i, o]: sigmoid(f) -> v=f*c starts
  while later gate matmuls still stream.
- h^T history kept in SBUF per-m-tile ring tiles (hTs); Xs phase reads
  its stationary operands straight from them (no DRAM round trip).
- bwd-direction X precompute emitted in descending x-mt order so the
  bwd recurrence (which consumes x reversed) starts immediately.
- ih/A/Xs/B emitted interleaved (software pipeline) so the Tile
  scheduler sees instructions in near-execution order.
- hh/ws weights fp8-e4m3 DoubleRow (0.5 cyc/row); ih bf16; gate
  activations and c-state fp16 (DVE 2x on the whole c-chain).
"""

import sys

sys.path.insert(0, "/opt/trn_rl_repo")

import numpy as np
import ml_dtypes

import concourse.bass as bass
import concourse.mybir as mybir
import concourse.tile as tile
from concourse import bacc
from concourse.bass_utils import run_bass_kernel_spmd

AF = mybir.ActivationFunctionType
BF16 = mybir.dt.bfloat16
F16 = mybir.dt.float16
F32 = mybir.dt.float32
F8 = mybir.dt.float8e4
DR = mybir.MatmulPerfMode.DoubleRow

B, D, H = 256, 256, 512
H2 = 2 * H
G1 = 4 * H           # 2048 = L1 gate width per direction
G2 = 4 * H2          # 4096 = L2 gate width
L = 2
NCORES = 8
BL = B // NCORES     # 32 local batch

# gate-quarter emission order: f first (so v=f*c starts early), o last
QORDER = (2, 0, 1, 3)  # col order is [g|i|f|o]


def _ih_iter(nc, pools, T, t_, g, i):
    """One ih iteration: Xf[i] (fwd) + Xb[MT-1-i] (bwd, descending)."""
    MT = T // 4
    xT = t_["xT"]
    ihx, ihp, iho = pools["ihx"], pools["ihp"], pools["iho"]
    for d in ("f", "b"):
        mt = i if d == "f" else MT - 1 - i
        t0 = mt * 4
        xt = ihx.tile([128, 2, 4, 32], BF16, tag=f"xt_{d}", name=f"xt_{d}")
        nc.sync.dma_start(
            out=xt,
            in_=xT.rearrange("(k p) t b -> p k t b", p=128)[:, :, t0:t0 + 4, :],
        )
        Xd = g["Xf"] if d == "f" else g["Xb"]
        wih = t_[f"w{d}_ih_sb"]
        bih = t_[f"b{d}_sb"]
        for nh in range(4):
            nsl = slice(nh * 512, (nh + 1) * 512)
            ps = ihp.tile([128, 512], F32, tag="ps", name="ps")
            for kt in range(2):
                nc.tensor.matmul(
                    ps, xt[:, kt].rearrange("p t b -> p (t b)"),
                    wih[:, kt, nsl], start=(kt == 0), stop=False,
                )
            nc.tensor.matmul(
                ps, t_["ones_t"][:, :128], bih[:, nsl],
                start=False, stop=True,
            )
            ot = iho.tile([128, 512], BF16, tag="ot", name="ot")
            nc.vector.tensor_copy(ot, ps)
            nc.sync.dma_start(
                out=Xd[mt][:, :, nsl].rearrange("t b n -> (t b) n"),
                in_=ot,
            )


def _a_step(nc, pools, T, t_, g, s):
    """One L1 step; fwd at cols 0:512, bwd at cols 512:1024 per quarter."""
    ax, ag, atr, ah, aact = (pools[k] for k in ("ax", "ag", "atr", "ah", "aact"))
    c1 = g["c1"]
    if s % 4 == 0:
        # [128, dir, kt, st, b] fp8 - per-m-tile h^T history ring
        g["hTs"][s // 4] = ah.tile(
            [128, 2, 4, 4, 32], F8, tag="hTs", name=f"hTs{s//4}")
    hTs = g["hTs"][s // 4]

    def prev_ap(bi, kp):
        if s == 0:
            return g["hT0"][:, bi, 2 * kp:2 * kp + 2, :]
        return g["hTs"][(s - 1) // 4][:, bi, 2 * kp:2 * kp + 2, (s - 1) % 4, :]

    xfb = ax.tile([32, 2 * G1], BF16, tag="xfb", name="xfb")
    nc.sync.dma_start(out=xfb[:, 0:G1], in_=g["Xf"][s // 4][s % 4])
    sb = T - 1 - s
    nc.sync.dma_start(out=xfb[:, G1:2 * G1], in_=g["Xb"][sb // 4][sb % 4])

    acts = {}
    for q in QORDER:
        nm = {2: "sf", 0: "tg", 1: "si", 3: "so"}[q]
        acts[nm] = aact.tile([32, 1024], F16, tag=nm, name=nm)
    for q in QORDER:
        nm = {2: "sf", 0: "tg", 1: "si", 3: "so"}[q]
        fn = AF.Tanh if q == 0 else AF.Sigmoid
        for bi, d in enumerate(("f", "b")):
            gp = ag.tile([32, 512], F32, tag="gq", name=f"gq{q}{d}")
            nc.tensor.matmul(
                gp, t_["id32_t"],
                xfb[:, bi * G1 + q * 512: bi * G1 + (q + 1) * 512],
                start=True, stop=False,
            )
            whh = t_[f"w{d}_hh_sb"]
            for kp in range(2):
                nc.tensor.matmul(
                    gp, prev_ap(bi, kp),
                    whh[:, 2 * kp:2 * kp + 2, q * 512:(q + 1) * 512],
                    start=False, stop=(kp == 1), perf_mode=DR,
                )
            nc.scalar.activation(
                acts[nm][:, 512 * bi:512 * bi + 512], gp, fn)
    sf, tg, si, so = acts["sf"], acts["tg"], acts["si"], acts["so"]

    v = aact.tile([32, 1024], F16, tag="v", name="v")
    nc.vector.tensor_mul(v, sf, c1)
    u = aact.tile([32, 1024], F16, tag="u", name="u")
    nc.vector.tensor_mul(u, si, tg)
    nc.vector.tensor_add(c1, u, v)
    tcl = aact.tile([32, 1024], F16, tag="tc", name="tc")
    nc.scalar.activation(tcl, c1, AF.Tanh)
    h = aact.tile([32, 1024], BF16, tag="h", name="h")
    nc.vector.tensor_mul(h, so, tcl)

    ptr = atr.tile([128, 8, 32], BF16, tag="ptr", name="ptr")
    for ci in range(8):  # cols 0:512 fwd (kt=ci), 512:1024 bwd (kt=ci-4)
        nc.tensor.transpose(
            ptr[:, ci], h[:, ci * 128:(ci + 1) * 128], t_["id32_t"],
        )
    nc.vector.tensor_copy(
        hTs[:, :, :, s % 4, :], ptr.rearrange("p (d k) b -> p d k b", d=2))


def _xs_mt(nc, pools, T, t_, g, mt):
    """Xs[mt] = combined @ Ws_ih^T + bs (natural col order)."""
    sp, so_ = pools["sp"], pools["so_"]
    wsih = t_["ws_ih_sb"]
    hTs = g["hTs"][mt]
    for chunk in range(8):
        csl = slice(chunk * 512, (chunk + 1) * 512)
        ps = sp.tile([128, 512], F32, tag="ps", name="ps")
        # stationary k-slab pairs: (f0,f1),(f2,f3),(b0,b1),(b2,b3)
        for kp in range(4):
            bi = kp // 2
            ko = (kp % 2) * 2
            lhs = hTs[:, bi, ko:ko + 2, :, :].rearrange("p k t b -> p k (t b)")
            nc.tensor.matmul(
                ps, lhs, wsih[:, 2 * kp:2 * kp + 2, csl],
                start=(kp == 0), stop=False, perf_mode=DR,
            )
        nc.tensor.matmul(
            ps, t_["ones_t"][:, :128], t_["bs_sb"][:, csl],
            start=False, stop=True,
        )
        ot = so_.tile([128, 512], BF16, tag="ot", name="ot")
        nc.vector.tensor_copy(ot, ps)
        nc.sync.dma_start(
            out=g["Xs"][mt][:, :, csl].rearrange("t b n -> (t b) n"),
            in_=ot,
        )


def _b_step(nc, pools, T, t_, g, s):
    """One L2 step; hidden halves at cols 0:512 / 512:1024 per quarter."""
    bx, bg, btr, bact = (pools[k] for k in ("bx", "bg", "btr", "bact"))
    wshh = t_["ws_hh_sb"]
    h2T, c2 = g["h2T"], g["c2"]
    xs2 = bx.tile([32, G2], BF16, tag="xs2", name="xs2")
    nc.sync.dma_start(out=xs2, in_=g["Xs"][s // 4][s % 4])
    acts = {}
    for q in QORDER:
        nm = {2: "sf2", 0: "tg2", 1: "si2", 3: "so2"}[q]
        acts[nm] = bact.tile([32, 1024], F16, tag=nm, name=nm)
    for q in QORDER:
        nm = {2: "sf2", 0: "tg2", 1: "si2", 3: "so2"}[q]
        fn = AF.Tanh if q == 0 else AF.Sigmoid
        for j in range(2):
            wsl = slice(1024 * q + 512 * j, 1024 * q + 512 * j + 512)
            gp = bg.tile([32, 512], F32, tag="gq2", name=f"gq2_{q}{j}")
            nc.tensor.matmul(
                gp, t_["id32_t"], xs2[:, wsl],
                start=True, stop=False,
            )
            for kp in range(4):
                nc.tensor.matmul(
                    gp, h2T[:, 2 * kp:2 * kp + 2, :],
                    wshh[:, 2 * kp:2 * kp + 2, wsl],
                    start=False, stop=(kp == 3), perf_mode=DR,
                )
            nc.scalar.activation(
                acts[nm][:, 512 * j:512 * j + 512], gp, fn)
    sf, tg, si, so = acts["sf2"], acts["tg2"], acts["si2"], acts["so2"]

    v = bact.tile([32, 1024], F16, tag="v2", name="v2")
    nc.vector.tensor_mul(v, sf, c2)
    u = bact.tile([32, 1024], F16, tag="u2", name="u2")
    nc.vector.tensor_mul(u, si, tg)
    nc.vector.tensor_add(c2, u, v)
    tcl = bact.tile([32, 1024], F16, tag="tc2", name="tc2")
    nc.scalar.activation(tcl, c2, AF.Tanh)
    h2 = bact.tile([32, 1024], BF16, tag="h2", name="h2")
    nc.vector.tensor_mul(h2, so, tcl)

    ptr = btr.tile([128, 8, 32], BF16, tag="ptr2", name="ptr2")
    for ci in range(8):  # cols are h2-dims in natural (slab) order
        nc.tensor.transpose(
            ptr[:, ci], h2[:, ci * 128:(ci + 1) * 128], t_["id32_t"],
        )
    nc.vector.tensor_copy(h2T, ptr)


def _classifier(nc, pools, t_, g):
    bg, bact = pools["bg"], pools["bact"]
    h2b = bact.tile([128, 8, 32], BF16, tag="h2b", name="h2b")
    nc.vector.tensor_copy(h2b, g["h2T"])
    ps_full = bg.tile([32, 512], F32, tag="gq2", name="ps_cls")
    ps_o = ps_full[0:BL, 0:L]
    for kt in range(8):
        nc.tensor.matmul(
            ps_o, h2b[:, kt], t_["wl_sb"][:, kt],
            start=(kt == 0), stop=False,
        )
    nc.tensor.matmul(
        ps_o, t_["ones_t"][:, :BL], t_["bl_sb"], start=False, stop=True)
    o_sb = bact.tile([BL, L], F32, tag="o_sb", name="o_sb")
    nc.scalar.activation(o_sb, ps_o, AF.Sigmoid)
    nc.sync.dma_start(out=t_["out"][:, :], in_=o_sb)


def _build_nc(T: int, use_fp8=True):
    assert use_fp8, "v3 kernel is fp8-DR only"
    nc = bacc.Bacc(None, target_bir_lowering=False)

    t_ = {}
    t_["xT"] = nc.dram_tensor("xT", [D, T, BL], BF16, kind="ExternalInput")
    for name, shape in (
        ("wf_ih", [D, G1]), ("wb_ih", [D, G1]),
        ("bf_r", [1, G1]), ("bb_r", [1, G1]), ("bs_r", [1, G2]),
        ("wl", [H2, L]), ("bl_r", [1, L]),
        ("ones_r", [1, 128]), ("id32", [32, 32]),
    ):
        t_[name] = nc.dram_tensor(name, shape, BF16, kind="ExternalInput")
    for name, shape in (
        ("wf_hh", [H, G1]), ("wb_hh", [H, G1]),
        ("ws_hh", [H2, G2]), ("ws_ih", [H2, G2]),
    ):
        t_[name] = nc.dram_tensor(name, shape, F8, kind="ExternalInput")
    t_["out"] = nc.dram_tensor("out", [BL, L], F32, kind="ExternalOutput")

    with tile.TileContext(nc) as tc:
        from contextlib import ExitStack
        with ExitStack() as ctx:
            ec = ctx.enter_context
            dram = ec(tc.tile_pool(name="dram", bufs=1, space="DRAM"))
            const = ec(tc.tile_pool(name="const", bufs=1))
            wpool = ec(tc.tile_pool(name="wpool", bufs=1))
            state = ec(tc.tile_pool(name="state", bufs=1))

            MT = T // 4
            g = {"hTs": {}}
            g["Xf"] = [dram.tile([4, BL, G1], BF16, tag=f"Xf{m}", name=f"Xf{m}") for m in range(MT)]
            g["Xb"] = [dram.tile([4, BL, G1], BF16, tag=f"Xb{m}", name=f"Xb{m}") for m in range(MT)]
            g["Xs"] = [dram.tile([4, BL, G2], BF16, tag=f"Xs{m}", name=f"Xs{m}") for m in range(MT)]

            ones_t = const.tile([1, 128], BF16, tag="ones_t", name="ones_t")
            nc.sync.dma_start(out=ones_t, in_=t_["ones_r"][:, :])
            id32_t = const.tile([32, 32], BF16, tag="id32_t", name="id32_t")
            nc.sync.dma_start(out=id32_t, in_=t_["id32"][:, :])
            t_["ones_t"], t_["id32_t"] = ones_t, id32_t

            # weights to SBUF
            for d in ("f", "b"):
                w = wpool.tile([128, 2, G1], BF16, tag=f"w{d}ih", name=f"w{d}ih")
                nc.sync.dma_start(out=w, in_=t_[f"w{d}_ih"].rearrange("(k p) n -> p k n", p=128))
                t_[f"w{d}_ih_sb"] = w
                w = wpool.tile([128, 4, G1], F8, tag=f"w{d}hh", name=f"w{d}hh")
                nc.sync.dma_start(out=w, in_=t_[f"w{d}_hh"].rearrange("(k p) n -> p k n", p=128))
                t_[f"w{d}_hh_sb"] = w
                bt = wpool.tile([1, G1], BF16, tag=f"b{d}", name=f"b{d}")
                nc.sync.dma_start(out=bt, in_=t_[f"b{d}_r"][:, :])
                t_[f"b{d}_sb"] = bt
            for nm in ("ws_ih", "ws_hh"):
                w = wpool.tile([128, 8, G2], F8, tag=nm, name=nm)
                nc.sync.dma_start(out=w, in_=t_[nm].rearrange("(k p) n -> p k n", p=128))
                t_[f"{nm}_sb"] = w
            bs = wpool.tile([1, G2], BF16, tag="bs", name="bs")
            nc.sync.dma_start(out=bs, in_=t_["bs_r"][:, :])
            t_["bs_sb"] = bs
            wl = wpool.tile([128, 8, L], BF16, tag="wl", name="wl")
            nc.sync.dma_start(out=wl, in_=t_["wl"].rearrange("(k p) n -> p k n", p=128))
            t_["wl_sb"] = wl
            bl = wpool.tile([1, L], BF16, tag="bl", name="bl")
            nc.sync.dma_start(out=bl, in_=t_["bl_r"][:, :])
            t_["bl_sb"] = bl

            g["hT0"] = state.tile([128, 2, 4, 32], F8, tag="hT0", name="hT0")
            nc.vector.memset(g["hT0"], 0.0)
            g["c1"] = state.tile([32, 1024], F16, tag="c1", name="c1")
            nc.vector.memset(g["c1"], 0.0)
            g["h2T"] = state.tile([128, 8, 32], F8, tag="h2T", name="h2T")
            nc.vector.memset(g["h2T"], 0.0)
            g["c2"] = state.tile([32, 1024], F16, tag="c2", name="c2")
            nc.vector.memset(g["c2"], 0.0)

            pools = {}
            pools["ihx"] = ec(tc.tile_pool(name="ihx", bufs=3))
            pools["ihp"] = ec(tc.tile_pool(name="ihp", bufs=1, space="PSUM"))
            pools["iho"] = ec(tc.tile_pool(name="iho", bufs=3))
            pools["ax"] = ec(tc.tile_pool(name="ax", bufs=2))
            pools["ag"] = ec(tc.tile_pool(name="ag", bufs=2, space="PSUM"))
            pools["atr"] = ec(tc.tile_pool(name="atr", bufs=1, space="PSUM"))
            pools["ah"] = ec(tc.tile_pool(name="ah", bufs=3))
            pools["aact"] = ec(tc.tile_pool(name="aact", bufs=1))
            pools["sp"] = ec(tc.tile_pool(name="sp", bufs=1, space="PSUM"))
            pools["so_"] = ec(tc.tile_pool(name="so_", bufs=3))
            pools["bx"] = ec(tc.tile_pool(name="bx", bufs=2))
            pools["bg"] = ec(tc.tile_pool(name="bg", bufs=2, space="PSUM"))
            pools["btr"] = ec(tc.tile_pool(name="btr", bufs=1, space="PSUM"))
            pools["bact"] = ec(tc.tile_pool(name="bact", bufs=1))

            # software pipeline: ih(i) || A(4(i-1)..) || Xs(i-2) || B(4(i-2)..)
            for i in range(MT + 2):
                if i < MT:
                    _ih_iter(nc, pools, T, t_, g, i)
                if 1 <= i <= MT:
                    for st in range(4):
                        _a_step(nc, pools, T, t_, g, 4 * (i - 1) + st)
                if 2 <= i:
                    mt = i - 2
                    _xs_mt(nc, pools, T, t_, g, mt)
                    for st in range(4):
                        _b_step(nc, pools, T, t_, g, 4 * mt + st)
            _classifier(nc, pools, t_, g)
    nc.compile()
    return nc


_NC_CACHE = {}
USE_FP8 = True


def _get_nc(T, use_fp8=None):
    if T not in _NC_CACHE:
        _NC_CACHE[T] = _build_nc(T)
    return _NC_CACHE[T]


def _bf16(a):
    return np.ascontiguousarray(np.asarray(a, dtype=np.float32)).astype(ml_dtypes.bfloat16)


def _f8(a):
    a = np.clip(np.asarray(a, dtype=np.float32), -240.0, 240.0)
    return np.ascontiguousarray(a).astype(ml_dtypes.float8_e4m3)


def _prep_weights(Wf_ih, Wf_hh, bf, Wb_ih, Wb_hh, bb, Ws_ih, Ws_hh, bs, Wl, bl):
    # gate reorder [i|f|g|o] -> [g|i|f|o]
    r1 = np.r_[2 * H:3 * H, 0:H, H:2 * H, 3 * H:4 * H]
    r2 = np.r_[2 * H2:3 * H2, 0:H2, H2:2 * H2, 3 * H2:4 * H2]
    return {
        "wf_ih": _bf16(np.asarray(Wf_ih)[r1].T),
        "wf_hh": _f8(np.asarray(Wf_hh)[r1].T),
        "wb_ih": _bf16(np.asarray(Wb_ih)[r1].T),
        "wb_hh": _f8(np.asarray(Wb_hh)[r1].T),
        "ws_ih": _f8(np.asarray(Ws_ih)[r2].T),
        "ws_hh": _f8(np.asarray(Ws_hh)[r2].T),
        "bf_r": _bf16(np.asarray(bf)[r1][None, :]),
        "bb_r": _bf16(np.asarray(bb)[r1][None, :]),
        "bs_r": _bf16(np.asarray(bs)[r2][None, :]),
        "wl": _bf16(np.asarray(Wl).T),
        "bl_r": _bf16(np.asarray(bl)[None, :]),
        "ones_r": _bf16(np.ones((1, 128), np.float32)),
        "id32": _bf16(np.eye(32, dtype=np.float32)),
    }


def input_specs(T):
    return [
        ("xT", [D, T, BL], BF16), ("wf_ih", [D, G1], BF16),
        ("wb_ih", [D, G1], BF16), ("bf_r", [1, G1], BF16),
        ("bb_r", [1, G1], BF16), ("bs_r", [1, G2], BF16),
        ("wl", [H2, L], BF16), ("bl_r", [1, L], BF16),
        ("ones_r", [1, 128], BF16), ("id32", [32, 32], BF16),
        ("wf_hh", [H, G1], F8), ("wb_hh", [H, G1], F8),
        ("ws_hh", [H2, G2], F8), ("ws_ih", [H2, G2], F8),
    ]


def make_in_maps(inputs):
    x = np.asarray(inputs["x"], dtype=np.float32)
    wmap = _prep_weights(
        inputs["Wf_ih"], inputs["Wf_hh"], inputs["bf"],
        inputs["Wb_ih"], inputs["Wb_hh"], inputs["bb"],
        inputs["Ws_ih"], inputs["Ws_hh"], inputs["bs"],
        inputs["Wl"], inputs["bl"])
    in_maps = []
    for c in range(NCORES):
        m = dict(wmap)
        m["xT"] = _bf16(x[c * BL:(c + 1) * BL].transpose(2, 1, 0))
        in_maps.append(m)
    return in_maps


def kernel(x, Wf_ih, Wf_hh, bf, Wb_ih, Wb_hh, bb, Ws_ih, Ws_hh, bs, Wl, bl):
    x = np.asarray(x, dtype=np.float32)
    T = x.shape[1]
    nc = _get_nc(T)
    in_maps = make_in_maps(dict(
        x=x, Wf_ih=Wf_ih, Wf_hh=Wf_hh, bf=bf, Wb_ih=Wb_ih, Wb_hh=Wb_hh,
        bb=bb, Ws_ih=Ws_ih, Ws_hh=Ws_hh, bs=bs, Wl=Wl, bl=bl))
    res = run_bass_kernel_spmd(nc, in_maps, list(range(NCORES)))
    return np.concatenate([res.results[c]["out"] for c in range(NCORES)], axis=0)


if __name__ == "__main__":
    rng = np.random.default_rng(0)
    T = int(sys.argv[1]) if len(sys.argv) > 1 else 8
    ins = {
        "x": rng.standard_normal((B, T, D), dtype=np.float32),
        "Wf_ih": rng.standard_normal((4 * H, D), dtype=np.float32) * 0.05,
        "Wf_hh": rng.standard_normal((4 * H, H), dtype=np.float32) * 0.04,
        "bf": np.zeros(4 * H, np.float32),
        "Wb_ih": rng.standard_normal((4 * H, D), dtype=np.float32) * 0.05,
        "Wb_hh": rng.standard_normal((4 * H, H), dtype=np.float32) * 0.04,
        "bb": np.zeros(4 * H, np.float32),
        "Ws_ih": rng.standard_normal((4 * H2, H2), dtype=np.float32) * 0.03,
        "Ws_hh": rng.standard_normal((4 * H2, H2), dtype=np.float32) * 0.03,
        "bs": np.zeros(4 * H2, np.float32),
        "Wl": rng.standard_normal((L, H2), dtype=np.float32) * 0.04,
        "bl": np.zeros(L, np.float32),
    }
    got = kernel(**ins)

    def sigmoid(z):
        return 1.0 / (1.0 + np.exp(-z))

    def scan(xs, Wih, Whh, bvec):
        Tn, Bn, _ = xs.shape
        Hh = Whh.shape[1]
        h = np.zeros((Bn, Hh), np.float32)
        c = np.zeros((Bn, Hh), np.float32)
        hs = []
        for t in range(Tn):
            gg = xs[t] @ Wih.T + h @ Whh.T + bvec
            i, f, ge, o = np.split(gg, 4, axis=-1)
            c = sigmoid(f) * c + sigmoid(i) * np.tanh(ge)
            h = sigmoid(o) * np.tanh(c)
            hs.append(h.copy())
        return np.stack(hs), h

    xs = np.swapaxes(ins["x"], 0, 1)
    fseq, _ = scan(xs, ins["Wf_ih"], ins["Wf_hh"], ins["bf"])
    bseq, _ = scan(xs[::-1], ins["Wb_ih"], ins["Wb_hh"], ins["bb"])
    comb = np.concatenate([fseq, bseq], -1)
    _, hs = scan(comb, ins["Ws_ih"], ins["Ws_hh"], ins["bs"])
    ref = sigmoid(hs @ ins["Wl"].T + ins["bl"])
    rel = np.abs(got - ref) / np.maximum(np.abs(ref), 1e-6)
    print(f"T={T}: max rel {rel.max():.3e}  mean rel {rel.mean():.3e}")
